# revision 3
# baseline (speedup 1.0000x reference)
# GQA attention block on 8 Trainium2 NeuronCores.
# Sharding: core = (batch b in {0,1}) x (tensor-parallel t in {0..3}).
# Each core: batch row b, 4 query heads {4t..4t+3}, 2 kv heads {2t, 2t+1}.
# W_Q/W_K/W_V split column-wise (per-head), W_O row-wise; the 4 TP partial
# outputs per batch are summed on the host (the "all-reduce").
import math
import sys

sys.path.insert(0, "/opt/trn_rl_repo")

import ml_dtypes
import numpy as np

import concourse.bacc as bacc
import concourse.bass as bass
import concourse.mybir as mybir
import concourse.tile as tile
from contextlib import ExitStack

BF = mybir.dt.bfloat16
F32 = mybir.dt.float32
bfnp = ml_dtypes.bfloat16

EMB = 2048
HEADS = 16
G = 2
HD = 128          # head dim
KV = HEADS // G   # 8 kv heads
B = 2
S = 2048
NCORES = 8
TP = 4
HQ = HEADS // TP       # 4 q heads per core
HKV = KV // TP         # 2 kv heads per core
NE = EMB // 128        # 16 contraction chunks
SC4 = S // 512         # 4 s-chunks of 512
SC16 = S // 128        # 16 s-chunks of 128
SCALE = 1.0 / math.sqrt(float(EMB))

_NC = None


def _build_program():
    nc = bacc.Bacc("TRN2", target_bir_lowering=False, debug=False)

    xT = nc.dram_tensor("xT", (EMB, S), BF, kind="ExternalInput")
    wq = nc.dram_tensor("wq", (EMB, HQ * HD), BF, kind="ExternalInput")
    wk = nc.dram_tensor("wk", (EMB, HKV * HD), BF, kind="ExternalInput")
    wv = nc.dram_tensor("wv", (EMB, HKV * HD), BF, kind="ExternalInput")
    wo = nc.dram_tensor("wo", (HQ * HD, EMB), BF, kind="ExternalInput")
    cosT = nc.dram_tensor("cosT", (HD, S), F32, kind="ExternalInput")
    sinT = nc.dram_tensor("sinT", (HD, S), F32, kind="ExternalInput")
    out = nc.dram_tensor("out", (S, EMB), F32, kind="ExternalOutput")

    with tile.TileContext(nc) as tc, ExitStack() as ctx:
        persist = ctx.enter_context(tc.tile_pool(name="persist", bufs=1))
        # qk_sb j-blocks: 0..3 = roped Q heads, 4..5 = roped K kv-heads; [d, s]
        qk_sb = persist.tile([128, HQ + HKV, S], BF)
        # V in [t, d] layout: [t_part, t_chunk, kvl*128+d]
        v_sb = persist.tile([128, SC16, HKV * HD], BF)
        ctx_sb = persist.tile([128, HQ, S], BF)      # [d, head, s]
        wo_sb = persist.tile([128, HQ, EMB], BF)     # [d, head, e_out]
        ones_sb = persist.tile([128, 1], BF)
        nc.vector.memset(ones_sb, 1.0)
        for jb in range(HQ):
            nc.sync.dma_start(out=wo_sb[:, jb, :], in_=wo[jb * 128:(jb + 1) * 128, :])

        # ---------------- Phase 1: projections + RoPE ----------------
        with tc.tile_pool(name="xt", bufs=1) as xt_pool, \
             tc.tile_pool(name="wts", bufs=1) as w_pool, \
             tc.tile_pool(name="ropet", bufs=4) as rope_t, \
             tc.tile_pool(name="pproj", bufs=8, space=bass.MemorySpace.PSUM) as pp:
            xt_sb = xt_pool.tile([128, NE, S], BF)
            for c in range(NE):
                nc.sync.dma_start(out=xt_sb[:, c, :], in_=xT[c * 128:(c + 1) * 128, :])
            wq_sb = w_pool.tile([128, NE, HQ * HD], BF)
            wk_sb = w_pool.tile([128, NE, HKV * HD], BF)
            wv_sb = w_pool.tile([128, NE, HKV * HD], BF)
            cos_sb = w_pool.tile([128, S], F32)
            sin_sb = w_pool.tile([128, S], F32)
            for c in range(NE):
                nc.sync.dma_start(out=wq_sb[:, c, :], in_=wq[c * 128:(c + 1) * 128, :])
                nc.sync.dma_start(out=wk_sb[:, c, :], in_=wk[c * 128:(c + 1) * 128, :])
                nc.sync.dma_start(out=wv_sb[:, c, :], in_=wv[c * 128:(c + 1) * 128, :])
            nc.sync.dma_start(out=cos_sb, in_=cosT[:, :])
            nc.sync.dma_start(out=sin_sb, in_=sinT[:, :])

            # Q (jb 0..3) and K (jb 4..5) in transposed [d, s] layout + RoPE
            for jb in range(HQ + HKV):
                pts = []
                for sc in range(SC4):
                    pts.append(pp.tile([128, 512], F32, tag="pts", name=f"pts_{jb}_{sc}"))
                for c in range(NE):
                    if jb < HQ:
                        lhsT = wq_sb[:, c, jb * 128:(jb + 1) * 128]
                    else:
                        kvl = jb - HQ
                        lhsT = wk_sb[:, c, kvl * 128:(kvl + 1) * 128]
                    for sc in range(SC4):
                        nc.tensor.matmul(
                            pts[sc], lhsT, xt_sb[:, c, sc * 512:(sc + 1) * 512],
                            start=(c == 0), stop=(c == NE - 1),
                        )
                for sc in range(SC4):
                    sl = slice(sc * 512, (sc + 1) * 512)
                    xs = rope_t.tile([128, 512], F32, tag="xs")
                    nc.scalar.copy(xs, pts[sc])
                    xw = rope_t.tile([128, 512], F32, tag="xw")
                    nc.sync.dma_start(out=xw[0:64, :], in_=xs[64:128, :])
                    nc.sync.dma_start(out=xw[64:128, :], in_=xs[0:64, :])
                    t1 = rope_t.tile([128, 512], F32, tag="t1")
                    nc.vector.tensor_mul(t1, xs, cos_sb[:, sl])
                    nc.vector.tensor_mul(xw, xw, sin_sb[:, sl])
                    nc.vector.tensor_add(qk_sb[:, jb, sl], t1, xw)

            # V in [t, d] layout (no rope): out[t=128, kvl*128+d]
            for st in range(SC16):
                pv = pp.tile([128, 512], F32, tag="pts")
                for c in range(NE):
                    nc.tensor.matmul(
                        pv[:, 0:HKV * HD],
                        xt_sb[:, c, st * 128:(st + 1) * 128],
                        wv_sb[:, c, :],
                        start=(c == 0), stop=(c == NE - 1),
                    )
                nc.scalar.copy(v_sb[:, st, :], pv[:, 0:HKV * HD])

        # ---------------- Phase 2: attention ----------------
        with tc.tile_pool(name="pscore", bufs=3, space=bass.MemorySpace.PSUM) as psc, \
             tc.tile_pool(name="pctx", bufs=2, space=bass.MemorySpace.PSUM) as pcx, \
             tc.tile_pool(name="pden", bufs=2, space=bass.MemorySpace.PSUM) as pdn, \
             tc.tile_pool(name="expp", bufs=4) as expp, \
             tc.tile_pool(name="misc", bufs=2) as misc:
            for h in range(HQ):
                kvjb = HQ + h // 2     # K block in qk_sb
                kvl = h // 2           # local kv index into v_sb columns
                for sc in range(SC4):
                    ssl = slice(sc * 512, (sc + 1) * 512)
                    cps = pcx.tile([128, 512], F32, tag="cps")
                    dps = pdn.tile([1, 512], F32, tag="dps")
                    for tcn in range(SC16):
                        sps = psc.tile([128, 512], F32, tag="sps")
                        nc.tensor.matmul(
                            sps,
                            qk_sb[:, kvjb, tcn * 128:(tcn + 1) * 128],
                            qk_sb[:, h, ssl],
                            start=True, stop=True,
                        )
                        ex = expp.tile([128, 512], BF, tag="ex")
                        nc.scalar.activation(
                            ex, sps, mybir.ActivationFunctionType.Exp, scale=SCALE
                        )
                        nc.tensor.matmul(
                            cps,
                            v_sb[:, tcn, kvl * 128:(kvl + 1) * 128],
                            ex,
                            start=(tcn == 0), stop=(tcn == SC16 - 1),
                        )
                        nc.tensor.matmul(
                            dps, ones_sb, ex,
                            start=(tcn == 0), stop=(tcn == SC16 - 1),
                        )
                    rc = misc.tile([1, 512], F32, tag="rc")
                    nc.vector.reciprocal(rc, dps)
                    rb = misc.tile([128, 512], F32, tag="rb")
                    nc.gpsimd.partition_broadcast(rb, rc)
                    nc.vector.tensor_mul(ctx_sb[:, h, ssl], cps, rb)

        # ---------------- Phase 3: output projection ----------------
        with tc.tile_pool(name="pout", bufs=4, space=bass.MemorySpace.PSUM) as pou, \
             tc.tile_pool(name="outs", bufs=4) as outp:
            for so in range(SC16):
                for ec in range(SC4):
                    ops = pou.tile([128, 512], F32, tag="ops")
                    for hl in range(HQ):
                        nc.tensor.matmul(
                            ops,
                            ctx_sb[:, hl, so * 128:(so + 1) * 128],
                            wo_sb[:, hl, ec * 512:(ec + 1) * 512],
                            start=(hl == 0), stop=(hl == HQ - 1),
                        )
                    ot = outp.tile([128, 512], F32, tag="ot")
                    nc.scalar.copy(ot, ops)
                    nc.sync.dma_start(
                        out=out[so * 128:(so + 1) * 128, ec * 512:(ec + 1) * 512],
                        in_=ot,
                    )

    nc.compile()
    return nc


def _get_nc():
    global _NC
    if _NC is None:
        _NC = _build_program()
    return _NC


def _rope_tables():
    half = HD // 2
    inv_freq = 1.0 / (10000.0 ** (np.arange(half, dtype=np.float64) * 2.0 / HD))
    ang = np.arange(S, dtype=np.float64)[:, None] * inv_freq[None, :]  # (S, 64)
    cos = np.concatenate([np.cos(ang), np.cos(ang)], axis=1).T  # (128, S)
    sin = np.concatenate([-np.sin(ang), np.sin(ang)], axis=1).T  # pre-signed
    return (np.ascontiguousarray(cos, dtype=np.float32),
            np.ascontiguousarray(sin, dtype=np.float32))


def build_in_maps(x, W_Q, W_K, W_V, W_O):
    x = np.asarray(x, dtype=np.float32)
    W_Q = np.asarray(W_Q, dtype=np.float32)
    W_K = np.asarray(W_K, dtype=np.float32)
    W_V = np.asarray(W_V, dtype=np.float32)
    W_O = np.asarray(W_O, dtype=np.float32)
    cos, sin = _rope_tables()
    in_maps = []
    xTb = [np.ascontiguousarray(x[b].T).astype(bfnp) for b in range(B)]
    for b in range(B):
        for t in range(TP):
            qheads = list(range(HQ * t, HQ * t + HQ))
            kvheads = [HKV * t + i for i in range(HKV)]
            idxq = [d * HEADS + h for h in qheads for d in range(HD)]
            idxkv = [d * KV + kv for kv in kvheads for d in range(HD)]
            rows_o = [h * HD + d for h in qheads for d in range(HD)]
            in_maps.append(dict(
                xT=xTb[b],
                wq=np.ascontiguousarray(W_Q[idxq, :].T).astype(bfnp),
                wk=np.ascontiguousarray(W_K[idxkv, :].T).astype(bfnp),
                wv=np.ascontiguousarray(W_V[idxkv, :].T).astype(bfnp),
                wo=np.ascontiguousarray(W_O[:, rows_o].T).astype(bfnp),
                cosT=cos,
                sinT=sin,
            ))
    return in_maps


def combine_outs(outs):
    out = np.empty((B, S, EMB), dtype=np.float32)
    for b in range(B):
        acc = outs[TP * b].astype(np.float32).copy()
        for t in range(1, TP):
            acc += outs[TP * b + t]
        out[b] = acc
    return out


LAST_RESULTS = None


def kernel(x, W_Q, W_K, W_V, W_O):
    global LAST_RESULTS
    from concourse.bass_utils import run_bass_kernel_spmd

    nc = _get_nc()
    in_maps = build_in_maps(x, W_Q, W_K, W_V, W_O)
    res = run_bass_kernel_spmd(nc, in_maps, list(range(NCORES)))
    LAST_RESULTS = res
    outs = [r["out"] for r in res.results]
    return combine_outs(outs)
